# revision 11
# baseline (speedup 1.0000x reference)
"""DialogueGCN fine-grained RGCN message passing on 8 Trainium2 cores.

Shapes (hardcoded): N=16 nodes, T=128 tokens, D=H=256 features, A=128 att dim,
E=N*N=256 edges. Edge (i,j) uses relation r = speaker[i]*32 + speaker[j]*2 + dir
(dir = 0 if i<j else 1) -- only 8 of the 512 Wrel1 matrices are ever touched
because speaker is binary, so the kernel ships just those 8.

Sharding: core c owns source nodes {2c, 2c+1} and their 32 edges. Each core
computes its local attention weights and per-edge messages, PSUM-accumulates
them per destination node in transposed [feature, token] layout, and a
ReduceScatter sums partials across cores while handing core c destination rows
{2c, 2c+1} (exactly its own nodes, so layer 2 and the final output need no
gathers). Host concatenates + transposes the 8 output shards.

v2: all message-path tensors, matmuls, and both ReduceScatter payloads run in
bf16 (fp32 PSUM accumulation; rel err ~7e-3 vs the 2e-2 gate). Attention
logits stay close to fp32 (bf16 inputs, fp32 softmax). Loads are spread over
four engine DMA queues in consumer order; speaker-masked source features are
precomputed host-side; root2 matmuls are hoisted before ReduceScatter 2.
"""

import contextlib
import ctypes
import sys
import types

import numpy as np

N = 16
T = 128
D = 256
A = 128
H = 256
NCORES = 8
NEG = -30000.0

_REL_IDS = [0, 1, 2, 3, 32, 33, 34, 35]  # slot a*4+b*2+d  <->  rel id 32a+2b+d

_CACHE = {}


def _install_profile_hook():
    """Make run_bass_kernel_spmd(trace=True) work under axon (missing antenv.axon_hooks)."""
    if "antenv.axon_hooks" in sys.modules:
        return
    try:
        import antenv
    except ImportError:
        return
    try:
        lib = ctypes.CDLL("/opt/axon/libaxon_pjrt.so")
    except OSError:
        return
    if not hasattr(lib, "axon_start_nrt_profile"):
        return
    lib.axon_start_nrt_profile.argtypes = [ctypes.POINTER(ctypes.c_int64), ctypes.c_size_t]
    lib.axon_start_nrt_profile.restype = ctypes.c_int64
    lib.axon_stop_nrt_profile.argtypes = [ctypes.c_char_p]
    lib.axon_stop_nrt_profile.restype = ctypes.c_int64

    @contextlib.contextmanager
    def _hook(output_dir, device_ids):
        import jax

        jax.devices()
        if device_ids:
            ids = (ctypes.c_int64 * len(device_ids))(*device_ids)
            rc = lib.axon_start_nrt_profile(ids, len(device_ids))
        else:
            rc = lib.axon_start_nrt_profile(None, 0)
        if rc != 0:
            raise RuntimeError(f"axon_start_nrt_profile rc={rc}")
        try:
            yield
        finally:
            n = lib.axon_stop_nrt_profile(str(output_dir).encode())
            print(f"profile: {n} file(s) written to {output_dir}", file=sys.stderr)

    mod = types.ModuleType("antenv.axon_hooks")
    mod.get_axon_ntff_profile_hook = lambda: _hook
    mod.set_axon_ntff_profile_hook = lambda h: None
    sys.modules["antenv.axon_hooks"] = mod
    antenv.axon_hooks = mod


def _build_program(ns0):
    import concourse.bacc as bacc
    import concourse.mybir as mybir
    import concourse.tile as tile

    F32 = mybir.dt.float32
    F32R = mybir.dt.float32r
    BF16 = mybir.dt.bfloat16
    EXP = mybir.ActivationFunctionType.Exp
    TANH = mybir.ActivationFunctionType.Tanh
    MULT = mybir.AluOpType.mult
    ADD = mybir.AluOpType.add

    nc = bacc.Bacc("TRN2", target_bir_lowering=False, debug=False, num_devices=NCORES)

    def din(name, shape, dt=F32):
        return nc.dram_tensor(name, shape, dt, kind="ExternalInput")

    # replicated inputs -- fp32 attention head, bf16 everything else
    gT = din("gT", [D, N], F32R)          # global_features.T
    wq = din("wq", [D, A], F32R)
    wk = din("wk", [D, A], F32R)
    vatt = din("vatt", [A, 1], F32R)
    xT = din("xT", [D, N * T], BF16)      # local_features.transpose(2,0,1) flat
    wb1 = din("wb1", [D, A], BF16)
    wb2 = din("wb2", [D, A], BF16)
    w8p = din("w8p", [D, 8 * H], BF16)    # Wrel1[_REL_IDS].transpose(1,0,2) flat
    wr1 = din("wr1", [D, H], BF16)
    w2 = din("w2", [H, H], BF16)          # Wrel2[0]
    wr2 = din("wr2", [H, H], BF16)
    kmrow = din("kmrow", [1, N * T], BF16)  # col j*T+s: 0 if s<len[j] else NEG
    ones1 = din("ones1", [1, T], BF16)
    b1c = din("b1c", [128, 2])            # b1 chunked: col oc = b1[oc*128 + p]
    b2c = din("b2c", [128, 2])
    # per-core inputs
    xsrcT = din("xsrcT", [D, 2 * T], BF16)  # xT columns of own 2 src nodes
    xsm = din("xsm", [D, 4 * T], BF16)    # col (k*2+a)*T+t: xT[d,i_k,t]*[spk==a]
    qm = din("qm", [T, 2])                # col k: 1 if t < len[i_k] else 0
    oh = din("oh", [N, 2 * T], F32R)      # cols [k*T:(k+1)*T] = onehot row i_k
    dirb = din("dirb", [T, 2 * N])        # col k*N+j: dir(i_k, j), bcast
    ident = din("ident", [128, 128], BF16)  # identity for PE transposes

    y = nc.dram_tensor("y", [2, H, T], F32, kind="ExternalOutput")

    rs1_in = nc.dram_tensor("rs1_in", [N, H, T], BF16)
    rs1_out = nc.dram_tensor("rs1_out", [2, H, T], BF16)
    rs2_in = nc.dram_tensor("rs2_in", [N, H, T], BF16)
    rs2_out = nc.dram_tensor("rs2_out", [2, H, T], BF16)
    gw_dram = nc.dram_tensor("gw_dram", [N * N], F32)
    ccw_in = nc.dram_tensor("ccw_in", [NCORES, 1], F32)
    ccw_out = nc.dram_tensor("ccw_out", [1, 1], F32)

    groups = [list(range(NCORES))]

    with tile.TileContext(nc) as tc, contextlib.ExitStack() as ctx:
        sb = ctx.enter_context(tc.tile_pool(name="sb", bufs=1))
        tr = ctx.enter_context(tc.tile_pool(name="tr", bufs=4))
        dma = nc.sync.dma_start
        mm = nc.tensor.matmul

        _dma_engs = [nc.sync, nc.scalar, nc.gpsimd]
        _dma_rr = [0]

        def ldma(out, in_):
            eng = _dma_engs[_dma_rr[0] % 3]
            _dma_rr[0] += 1
            eng.dma_start(out, in_)

        def load(src, dt=F32):
            t = sb.tile(list(src.shape), dt, tag="ld_" + src.name, name="ld_" + src.name)
            ldma(t[:, :], src[:, :])
            return t

        def load2(src, dt, tag):
            lo = sb.tile([128, src.shape[1]], dt, tag=tag + "lo", name=tag + "lo")
            hi = sb.tile([128, src.shape[1]], dt, tag=tag + "hi", name=tag + "hi")
            ldma(lo[:, :], src[0:128, :])
            ldma(hi[:, :], src[128:256, :])
            return lo, hi

        # loads in consumer order: A deps, D deps, B deps, C/Gb deps, late
        gt = load2(gT, F32R, "gt")
        wqc = load2(wq, F32R, "wq")
        wkc = load2(wk, F32R, "wk")
        tvatt = load(vatt, F32R)
        toh = load(oh, F32R)
        tdirb = load(dirb)
        xs = load2(xsm, BF16, "xsm")
        w8 = load2(w8p, BF16, "w8")
        wb2c = load2(wb2, BF16, "wb2")
        xt = load2(xT, BF16, "xt")
        wb1c = load2(wb1, BF16, "wb1")
        xsc = load2(xsrcT, BF16, "xsrc")
        tkmrow = load(kmrow, BF16)
        tones = load(ones1, BF16)
        tqm = load(qm)
        wr1c = load2(wr1, BF16, "wr1")
        w2c = load2(w2, BF16, "w2")
        tb1c = load(b1c)
        tb2c = load(b2c)
        wr2c = load2(wr2, BF16, "wr2")
        tid = load(ident, BF16)

        # warm the collective stream early: RS1 otherwise pays ~11us first-op cost
        nc.gpsimd.collective_compute("ReduceScatter", ADD, replica_groups=groups,
                                     ins=[ccw_in.ap().opt()], outs=[ccw_out.ap().opt()])

        # --- phase A: global attention -> per-edge scale vectors s[b][d] ---
        with tc.tile_pool(name="pA", bufs=1, space="PSUM") as pA:
            mps = pA.tile([A, N * N], F32, tag="mps")
            m3 = mps[:, :].rearrange("a (i j) -> a i j", i=N)
            mm(m3, wqc[0][:, :], gt[0][:, :].unsqueeze(2).broadcast_to([128, N, N]), start=True, stop=False)
            mm(m3, wqc[1][:, :], gt[1][:, :].unsqueeze(2).broadcast_to([128, N, N]), start=False, stop=False)
            mm(m3, wkc[0][:, :], gt[0][:, :].unsqueeze(1).broadcast_to([128, N, N]), start=False, stop=False)
            mm(m3, wkc[1][:, :], gt[1][:, :].unsqueeze(1).broadcast_to([128, N, N]), start=False, stop=True)
            tanhm = sb.tile([A, N * N], F32R, tag="tanhm")
            nc.scalar.activation(tanhm[:, :], mps[:, :], TANH)
            srowps = pA.tile([1, N * N], F32, tag="srowps")
            mm(srowps[:, :], tvatt[:, :], tanhm[:, :], start=True, stop=True)
            srow = sb.tile([1, N * N], F32, tag="srow")
            nc.vector.tensor_copy(srow[:, :], srowps[:, :])
            dma(gw_dram[:], srow[0:1, :])
            s16 = sb.tile([N, N], F32, tag="s16")
            dma(s16[:, :], gw_dram.ap().rearrange("(i j) -> i j", i=N))
            e16 = sb.tile([N, N], F32, tag="e16")
            den16 = sb.tile([N, 1], F32, tag="den16")
            nc.scalar.activation(e16[:, :], s16[:, :], EXP, accum_out=den16[:, :])
            rec16 = sb.tile([N, 1], F32, tag="rec16")
            nc.vector.reciprocal(rec16[:, :], den16[:, :])
            gw16 = sb.tile([N, N], F32R, tag="gw16")
            nc.vector.tensor_scalar(gw16[:, :], e16[:, :], rec16[:, :], None, MULT)
            gwps = pA.tile([T, 2 * N], F32, tag="gwps")
            mm(gwps[:, 0:N], toh[:, 0:T], gw16[:, :], start=True, stop=True)
            mm(gwps[:, N:2 * N], toh[:, T:2 * T], gw16[:, :], start=True, stop=True)
            gwb = sb.tile([T, 2 * N], F32, tag="gwb")  # col k*N+j = gw[i_k, j]
            nc.vector.tensor_copy(gwb[:, :], gwps[:, :])

        sdir = {}
        s_d1 = sb.tile([T, 2 * N], F32, tag="sd1")
        s_d0 = sb.tile([T, 2 * N], F32, tag="sd0")
        nc.vector.tensor_tensor(s_d1[:, :], gwb[:, :], tdirb[:, :], MULT)
        nc.vector.tensor_sub(s_d0[:, :], gwb[:, :], s_d1[:, :])
        sdir[1] = s_d1
        sdir[0] = s_d0

        # --- phase D: XWsum[k, beta] slabs from host-masked xsm ---
        xws = [sb.tile([T, 4 * H], BF16, tag=f"xws{k}", name=f"xws{k}") for k in (0, 1)]
        with tc.tile_pool(name="pD", bufs=2, space="PSUM") as pD:
            for k in (0, 1):
                for beta in range(4):
                    xwps = pD.tile([T, H], F32, tag="xwps")
                    first = True
                    for a in (0, 1):
                        r = a * 4 + beta
                        for chunk in (0, 1):
                            mm(xwps[:, :], xs[chunk][:, (k * 2 + a) * T:(k * 2 + a + 1) * T],
                               w8[chunk][:, r * H:(r + 1) * H],
                               start=first, stop=(a == 1 and chunk == 1))
                            first = False
                    eng = nc.scalar if beta % 2 == 0 else nc.vector
                    eng_copy = nc.scalar.copy if beta % 2 == 0 else nc.vector.tensor_copy
                    eng_copy(xws[k][:, beta * H:(beta + 1) * H], xwps[:, :])

        # --- phase B: p2T (all nodes), p1T (own srcs) ---
        p2t = sb.tile([A, N * T], BF16, tag="p2t")
        with tc.tile_pool(name="pB", bufs=2, space="PSUM") as pB:
            for q in range(4):
                p2ps = pB.tile([A, 512], F32, tag="p2ps")
                mm(p2ps[:, :], wb2c[0][:, :], xt[0][:, q * 512:(q + 1) * 512], start=True, stop=False)
                mm(p2ps[:, :], wb2c[1][:, :], xt[1][:, q * 512:(q + 1) * 512], start=False, stop=True)
                nc.scalar.copy(p2t[:, q * 512:(q + 1) * 512], p2ps[:, :])
            p1ps = pB.tile([A, 2 * T], F32, tag="p1ps")
            mm(p1ps[:, :], wb1c[0][:, :], xsc[0][:, :], start=True, stop=False)
            mm(p1ps[:, :], wb1c[1][:, :], xsc[1][:, :], start=False, stop=True)
            p1t = sb.tile([A, 2 * T], BF16, tag="p1t")
            nc.vector.tensor_copy(p1t[:, :], p1ps[:, :])

        # --- phase C: S + rank-1 key mask, exp, per-dst softmax scales, lw ---
        es = [sb.tile([T, N * T], BF16, tag=f"es{k}", name=f"es{k}") for k in (0, 1)]
        lw = [sb.tile([T, N * T], BF16, tag=f"lw{k}", name=f"lw{k}") for k in (0, 1)]
        lwscf = sb.tile([T, 2 * N], F32, tag="lwscf")
        with tc.tile_pool(name="pC", bufs=3, space="PSUM") as pC:
            for k in (0, 1):
                for g in range(4):
                    sps = pC.tile([T, 512], F32, tag="sps")
                    mm(sps[:, :], p1t[:, k * T:(k + 1) * T],
                       p2t[:, g * 512:(g + 1) * 512], start=True, stop=False)
                    mm(sps[:, :], tones[:, :], tkmrow[:, g * 512:(g + 1) * 512],
                       start=False, stop=True)
                    esg = es[k][:, g * 512:(g + 1) * 512]
                    nc.scalar.activation(esg, sps[:, :], EXP)
                    den = tr.tile([T, 4], F32, tag="den", bufs=4)
                    nc.vector.tensor_reduce(den[:, :], esg.rearrange("p (j s) -> p j s", j=4),
                                            mybir.AxisListType.X, ADD)
                    rec = tr.tile([T, 4], F32, tag="rec", bufs=4)
                    nc.vector.reciprocal(rec[:, :], den[:, :])
                    nc.vector.tensor_scalar(lwscf[:, k * N + g * 4:k * N + g * 4 + 4],
                                            rec[:, :], tqm[:, k:k + 1], None, MULT)

        # --- phase Gb (pre-RS1): root part of x1^T ---
        x1bt = sb.tile([128, 512], BF16, tag="x1bt")  # col (oc*2+k)*128+t
        with tc.tile_pool(name="pGb", bufs=1, space="PSUM") as pGb:
            rt1 = [pGb.tile([128, 2 * T], F32, tag=f"rt1{oc}", name=f"rt1{oc}") for oc in (0, 1)]
            for oc in (0, 1):
                mm(rt1[oc][:, :], wr1c[0][:, oc * 128:(oc + 1) * 128], xsc[0][:, :],
                   start=True, stop=False)
                mm(rt1[oc][:, :], wr1c[1][:, oc * 128:(oc + 1) * 128], xsc[1][:, :],
                   start=False, stop=True)
            for k in (0, 1):
                for oc in (0, 1):
                    nc.vector.tensor_scalar(
                        x1bt[:, (oc * 2 + k) * 128:(oc * 2 + k + 1) * 128],
                        rt1[oc][:, k * T:(k + 1) * T], tb1c[:, oc:oc + 1], None, ADD)

        # combined per-edge scale vectors for layer-1 ew: lwsc * gw * mask(dir)
        # (speaker masking is static: host sorts nodes so spk_j = [pos >= ns0])
        ewsc = {}
        for dd in (0, 1):
            t_ = sb.tile([T, 2 * N], F32, tag=f"ewsc{dd}", name=f"ewsc{dd}")
            nc.vector.tensor_tensor(t_[:, :], lwscf[:, :], sdir[dd][:, :], MULT)
            ewsc[dd] = t_

        # --- phase E: layer-1 messages; stationary XW slab, stream 4-dst ew tiles ---
        agg1sb = [sb.tile([128, 2048], BF16, tag=f"agg1sb{oc}", name=f"agg1sb{oc}")
                  for oc in (0, 1)]
        with tc.tile_pool(name="pE", bufs=1, space="PSUM") as pE:
            aggb = [[pE.tile([128, 512], F32, tag=f"aggb{oc}{g}", name=f"aggb{oc}{g}")
                     for g in range(4)] for oc in (0, 1)]
            def bank_ranges(g):
                lo, hi = 4 * g, 4 * g + 4
                r = []
                if lo < ns0:
                    r.append((lo, min(ns0, hi), 0))
                if hi > ns0:
                    r.append((max(ns0, lo), hi, 1))
                return r
            combos = [(k, dd) for k in (0, 1) for dd in (0, 1)]
            nmm = {g: len(bank_ranges(g)) * len(combos) for g in range(4)}
            seen = {}
            for it, (k, dd) in enumerate(combos):
                ew = tr.tile([T, N * T], BF16, tag="ew", bufs=3, name="ew")
                eng = nc.vector if it % 2 == 0 else nc.gpsimd
                eng.tensor_tensor(ew[:, :].rearrange("p (j s) -> p j s", j=N),
                                        es[k][:, :].rearrange("p (j s) -> p j s", j=N),
                                        ewsc[dd][:, k * N:(k + 1) * N]
                                        .unsqueeze(2).broadcast_to([T, N, T]), MULT)
                for oc in (0, 1):
                    for g in range(4):
                        for (jlo, jhi, b) in bank_ranges(g):
                            beta = b * 2 + dd
                            c0, c1 = (jlo - 4 * g) * T, (jhi - 4 * g) * T
                            idx = seen.get((oc, g), 0)
                            seen[(oc, g)] = idx + 1
                            mm(aggb[oc][g][:, c0:c1],
                               xws[k][:, beta * H + oc * 128:beta * H + oc * 128 + 128],
                               ew[:, g * 512 + c0:g * 512 + c1],
                               start=(idx == 0), stop=(idx == nmm[g] - 1))
            copy_engs = [nc.scalar.copy, nc.vector.tensor_copy]
            for oc in (0, 1):
                for g in range(4):
                    copy_engs[(oc * 4 + g) % 2](
                        agg1sb[oc][:, g * 512:(g + 1) * 512], aggb[oc][g][:, :])
                dst = rs1_in.ap().rearrange("j o s -> o j s")[oc * 128:(oc + 1) * 128, :, :]
                eng = nc.sync if oc == 0 else nc.scalar
                eng.dma_start(dst, agg1sb[oc][:, :].rearrange("p (j s) -> p j s", j=N))

        # --- ReduceScatter 1: layer-1 aggregate (transposed layout, bf16) ---
        nc.gpsimd.collective_compute("ReduceScatter", ADD, replica_groups=groups,
                                     ins=[rs1_in.ap().opt()], outs=[rs1_out.ap().opt()])

        # --- lw builds (overlap RS1): full layer-2 edge weights from es ---
        for k in (0, 1):
            eng = nc.vector if k == 0 else nc.gpsimd
            eng.tensor_tensor(lw[k][:, :].rearrange("p (j s) -> p j s", j=N),
                              es[k][:, :].rearrange("p (j s) -> p j s", j=N),
                              lwscf[:, k * N:(k + 1) * N]
                              .unsqueeze(2).broadcast_to([T, N, T]), MULT)

        # --- phase Ga (post-RS1): RS part of x1^T ---
        x1at = sb.tile([128, 512], BF16, tag="x1at")
        for k in (0, 1):
            eng = nc.sync if k == 0 else nc.scalar
            eng.dma_start(x1at[:, :].rearrange("p (oc k t) -> p oc k t", oc=2, k=2)[:, :, k, :],
                rs1_out[k].rearrange("(oc p) t -> p oc t", oc=2))
        x1t = sb.tile([128, 512], BF16, tag="x1t")
        nc.vector.tensor_add(x1t[:, :], x1bt[:, :], x1at[:, :])

        # --- x1 transposed to token-partition layout for the Z matmuls ---
        x1T = [sb.tile([128, 256], BF16, tag=f"x1T{k}", name=f"x1T{k}") for k in (0, 1)]
        with tc.tile_pool(name="pT", bufs=1, space="PSUM") as pT:
            x1Tps = [pT.tile([128, 256], BF16, tag=f"x1Tps{k}", name=f"x1Tps{k}")
                     for k in (0, 1)]
            for k in (0, 1):
                for oc in (0, 1):
                    nc.tensor.transpose(x1Tps[k][:, oc * 128:(oc + 1) * 128],
                                        x1t[:, (oc * 2 + k) * 128:(oc * 2 + k + 1) * 128],
                                        tid[:, :])
                nc.scalar.copy(x1T[k][:, :], x1Tps[k][:, :])

        # --- phase Ia (post-RS1): layer-2 Z = lw^T @ x1 (W2 deferred past RS2) ---
        with tc.tile_pool(name="pIa", bufs=1, space="PSUM") as pIa:
            agg2a = [[pIa.tile([128, 512], F32, tag=f"agg2a{oc}{g}", name=f"agg2a{oc}{g}")
                      for g in range(4)] for oc in (0, 1)]
            for k in (0, 1):
                for oc in (0, 1):
                    for g in range(4):
                        mm(agg2a[oc][g][:, :],
                           x1T[k][:, oc * 128:(oc + 1) * 128],
                           lw[k][:, g * 512:(g + 1) * 512],
                           start=(k == 0), stop=(k == 1))
            cp2t = [sb.tile([128, 2048], BF16, tag=f"cp2t{oc}", name=f"cp2t{oc}")
                    for oc in (0, 1)]
            for oc in (0, 1):
                for g in range(4):
                    copy_engs[g % 2](cp2t[oc][:, g * 512:(g + 1) * 512],
                                     agg2a[oc][g][:, :])
                dst = rs2_in.ap().rearrange("j o s -> o j s")[oc * 128:(oc + 1) * 128, :, :]
                eng = nc.sync if oc == 0 else nc.scalar
                eng.dma_start(dst, cp2t[oc][:, :].rearrange("p (j s) -> p j s", j=N))

        # --- ReduceScatter 2 (bf16) ---
        nc.gpsimd.collective_compute("ReduceScatter", ADD, replica_groups=groups,
                                     ins=[rs2_in.ap().opt()], outs=[rs2_out.ap().opt()])

        # --- phase K: y = Zsum @ W2 + x1 @ Wroot2 + b2 ---
        rsx = tr.tile([128, 512], BF16, tag="rsx", bufs=1, name="rsx")
        for k in (0, 1):
            eng = nc.sync if k == 0 else nc.scalar
            eng.dma_start(rsx[:, :].rearrange("p (oc k t) -> p oc k t", oc=2, k=2)[:, :, k, :],
                rs2_out[k].rearrange("(oc p) t -> p oc t", oc=2))
        ysb = tr.tile([128, 512], F32, tag="ysb", bufs=1, name="ysb")
        with tc.tile_pool(name="pK", bufs=1, space="PSUM") as pK:
            yps = [pK.tile([128, 2 * T], F32, tag=f"yps{oc2}", name=f"yps{oc2}")
                   for oc2 in (0, 1)]
            for oc2 in (0, 1):
                mm(yps[oc2][:, :], wr2c[0][:, oc2 * 128:(oc2 + 1) * 128],
                   x1t[:, 0:256], start=True, stop=False)
                mm(yps[oc2][:, :], wr2c[1][:, oc2 * 128:(oc2 + 1) * 128],
                   x1t[:, 256:512], start=False, stop=False)
                mm(yps[oc2][:, :], w2c[0][:, oc2 * 128:(oc2 + 1) * 128],
                   rsx[:, 0:256], start=False, stop=False)
                mm(yps[oc2][:, :], w2c[1][:, oc2 * 128:(oc2 + 1) * 128],
                   rsx[:, 256:512], start=False, stop=True)
                nc.vector.tensor_scalar(
                    ysb[:, oc2 * 256:(oc2 + 1) * 256],
                    yps[oc2][:, :], tb2c[:, oc2:oc2 + 1], None, ADD)
        for k in (0, 1):
            eng = nc.sync if k == 0 else nc.scalar
            eng.dma_start(y[k].rearrange("(oc p) t -> p oc t", oc=2),
                ysb[:, :].rearrange("p (oc k2 t) -> p oc k2 t", oc=2, k2=2)[:, :, k, :])

    nc.compile()
    return nc


def _get_program(ns0):
    key = ("nc", ns0)
    if key not in _CACHE:
        _CACHE[key] = _build_program(ns0)
    return _CACHE[key]


def _prep_inputs(global_features, local_features, speaker, length,
                 Wq, Wk, v_att, Wb1, Wb2, Wrel1, Wroot1, b1, Wrel2, Wroot2, b2,
                 perm):
    import ml_dtypes

    f = np.float32
    bf = ml_dtypes.bfloat16
    speaker = np.asarray(speaker).astype(np.int64)
    length = np.asarray(length).astype(np.int64)
    x = np.asarray(local_features, dtype=f)[perm]                # [N,T,D] sorted by spk
    xTf = np.ascontiguousarray(x.transpose(2, 0, 1).reshape(D, N * T))

    common = {
        "xT": xTf.astype(bf),
        "gT": np.ascontiguousarray(np.asarray(global_features, f)[perm].T),
        "wq": np.ascontiguousarray(np.asarray(Wq, f)),
        "wk": np.ascontiguousarray(np.asarray(Wk, f)),
        "vatt": np.ascontiguousarray(np.asarray(v_att, f).reshape(A, 1)),
        "wb1": np.ascontiguousarray(np.asarray(Wb1, f)).astype(bf),
        "wb2": np.ascontiguousarray(np.asarray(Wb2, f)).astype(bf),
        "w8p": np.ascontiguousarray(
            np.asarray(Wrel1, f)[_REL_IDS].transpose(1, 0, 2).reshape(D, 8 * H)).astype(bf),
        "wr1": np.ascontiguousarray(np.asarray(Wroot1, f)).astype(bf),
        "w2": np.ascontiguousarray(np.asarray(Wrel2, f)[0]).astype(bf),
        "wr2": np.ascontiguousarray(np.asarray(Wroot2, f)).astype(bf),
        "b1c": np.ascontiguousarray(np.asarray(b1, f).reshape(2, 128).T),
        "b2c": np.ascontiguousarray(np.asarray(b2, f).reshape(2, 128).T),
        "ones1": np.ones((1, T), bf),
        "ident": np.eye(128, dtype=bf),
    }
    pos = np.arange(T)
    lengthp = length[perm]
    kmask = np.where(pos[None, :] < lengthp[:, None], 0.0, NEG).astype(f)   # [N,T] perm'd
    common["kmrow"] = np.ascontiguousarray(kmask.reshape(1, N * T)).astype(bf)

    xT3 = xTf.reshape(D, N, T)
    in_maps = []
    for c in range(NCORES):
        p0, p1 = 2 * c, 2 * c + 1                 # positions owned by this core
        i0, i1 = int(perm[p0]), int(perm[p1])     # original node ids
        m = dict(common)
        m["xsrcT"] = np.ascontiguousarray(
            xT3[:, [p0, p1], :].reshape(D, 2 * T)).astype(bf)
        xsmv = np.zeros((D, 4, T), f)
        for k, (pk, ik) in enumerate(((p0, i0), (p1, i1))):
            xsmv[:, k * 2 + int(speaker[ik]), :] = xT3[:, pk, :]
        m["xsm"] = np.ascontiguousarray(xsmv.reshape(D, 4 * T)).astype(bf)
        qmv = np.stack([(pos < length[i0]), (pos < length[i1])], 1).astype(f)
        m["qm"] = np.ascontiguousarray(qmv)
        ohv = np.zeros((N, 2 * T), f)
        ohv[p0, 0:T] = 1.0
        ohv[p1, T:2 * T] = 1.0
        m["oh"] = ohv
        dirv = np.zeros((2, N), f)
        for k, ik in ((0, i0), (1, i1)):
            for jp in range(N):
                dirv[k, jp] = 0.0 if ik < int(perm[jp]) else 1.0
        m["dirb"] = np.ascontiguousarray(np.broadcast_to(dirv.reshape(1, 2 * N), (T, 2 * N)))
        in_maps.append(m)
    return in_maps


def _assemble(results, perm):
    out = np.empty((N, T, H), np.float32)
    for c in range(NCORES):
        shard = results[c]["y"]              # [2, H, T]
        out[int(perm[2 * c])] = shard[0].T
        out[int(perm[2 * c + 1])] = shard[1].T
    return out


def _perm_for(speaker):
    sp = np.asarray(speaker).astype(np.int64)
    perm = np.argsort(sp, kind="stable")
    return perm, int((sp == 0).sum())


def kernel(**inputs) -> np.ndarray:
    from concourse import bass_utils

    perm, ns0 = _perm_for(inputs["speaker"])
    nc = _get_program(ns0)
    in_maps = _prep_inputs(**inputs, perm=perm)
    res = bass_utils.run_bass_kernel_spmd(nc, in_maps, core_ids=list(range(NCORES)))
    return _assemble(res.results, perm)


def kernel_traced(**inputs):
    """Like kernel() but also returns BassKernelResults with an NTFF profile."""
    import tempfile

    from concourse import bass_utils

    _install_profile_hook()
    perm, ns0 = _perm_for(inputs["speaker"])
    nc = _get_program(ns0)
    in_maps = _prep_inputs(**inputs, perm=perm)
    tdir = tempfile.mkdtemp(prefix="dgcn_trace_")
    res = bass_utils.run_bass_kernel_spmd(nc, in_maps, core_ids=list(range(NCORES)),
                                          trace=True, tmpdir=tdir)
    return _assemble(res.results, perm), res
